# revision 23
# baseline (speedup 1.0000x reference)
"""MultiHeadSelfAttentionWithLagBias on 8 TRN2 NeuronCores.

Sharding: tensor-parallel over heads — 16 heads / 8 cores = 2 heads per
core. Each core computes QKV projections for its head slice (full x),
attention with the lag bias for its 2 heads over both batch elements,
and a partial output projection (its 128 rows of wo). Host sums the 8
partials and adds bo.

Device layout (per core):
  xT      (1024, 4096)  x transposed, tok = b*2048 + s
  QT/KT   (128, 4096)   q^T/k^T, partitions = [h0 dk(64) | h1 dk(64)]
  V       (128, 32, 130) per 128-tok chunk: [V_h0(64) | 1 | V_h1(64) | 1]
  scores  computed transposed: S^T (k on partitions, q on free) so the
          softmax denominator falls out of the PV matmul via the ones
          column, and O^T is produced in exactly the layout the output
          projection needs as its stationary operand.
  bias    B_h (2048, 2048) host-precomputed (symmetric), DMA-streamed.
"""

import ml_dtypes
import numpy as np
from contextlib import ExitStack

import concourse.bass as bass
import concourse.bacc as bacc
import concourse.mybir as mybir
import concourse.tile as tile
from concourse.bass_utils import run_bass_kernel_spmd
from concourse.masks import make_identity

F32 = mybir.dt.float32
F32R = mybir.dt.float32r
BF16 = mybir.dt.bfloat16
AF = mybir.ActivationFunctionType

# float32r: single-pass fp32 matmul mode (1 cycle/row vs 4 for fp32),
# ~1.2e-4 relative rounding on HW. Toggle for a full-precision fallback.
USE_F32R = True
MMDT = F32R if USE_F32R else F32

N_CORES = 8
B, S, D = 2, 2048, 1024
H, DK = 16, 64
TOK = B * S              # 4096
NQ = 512                 # q-chunk (matmul free dim)
NQC = S // NQ            # 4 q-chunks per batch
NJ = S // 128            # 16 k-chunks per batch
DCH = D // 128           # 8 contraction chunks

# Set by test.py for profiling; harness leaves these untouched.
TRACE = False
TRACE_DIR = None

_CACHED_NC = None


def _body(ctx: ExitStack, tc, aps):
    nc = tc.nc
    xT, wq, wk, wv, bq, bk, bv, wo, B0, B1, out = (
        aps["xT"], aps["wq"], aps["wk"], aps["wv"], aps["bq"], aps["bk"],
        aps["bv"], aps["wo"], aps["B0"], aps["B1"], aps["out"])
    Bh = [B0, B1]

    const = ctx.enter_context(tc.tile_pool(name="const", bufs=1))
    persist = ctx.enter_context(tc.tile_pool(name="persist", bufs=1))
    spool = ctx.enter_context(tc.tile_pool(name="spsum", bufs=2, space="PSUM"))
    opool = ctx.enter_context(tc.tile_pool(name="opsum", bufs=4, space="PSUM"))

    # ---- constants ----
    ident = const.tile([128, 128], F32, tag="id")
    make_identity(nc, ident[:])
    w_sb = {}
    for name, ap in (("q", wq), ("k", wk), ("v", wv)):
        t = const.tile([128, DCH, 128], MMDT, tag=f"w{name}")
        nc.sync.dma_start(t[:], ap.rearrange("(c p) m -> p c m", p=128))
        w_sb[name] = t
    b_sb = {}
    for name, ap in (("q", bq), ("k", bk), ("v", bv)):
        t = const.tile([128, 1], F32, tag=f"b{name}")
        nc.sync.dma_start(t[:], ap[:])
        b_sb[name] = t
    # wo split into the two 64-row halves so both out-proj matmuls run at
    # partition base 0.
    wo0 = const.tile([64, D], MMDT, tag="wo0")
    wo1 = const.tile([64, D], MMDT, tag="wo1")
    nc.sync.dma_start(wo0[:], wo[0:64, :])
    nc.sync.dma_start(wo1[:], wo[64:128, :])
    # ones row at partition 64 for broadcasting the softmax reciprocal
    # (gpsimd partition_broadcast is broken on HW via this exec path)
    ones64 = const.tile([65, 64], F32, tag="ones64")
    nc.vector.memset(ones64[:], 1.0)
    # fp32 ones staged for the f32r ones-columns of Vb (memset can't
    # write f32r; ACT copy is a rounding producer)
    ones_f32 = const.tile([128, 64], F32, tag="ones_f32")
    nc.vector.memset(ones_f32[:], 1.0)

    # ---- persistent activations ----
    QT = persist.tile([128, TOK], MMDT, tag="QT")
    KT = persist.tile([128, TOK], MMDT, tag="KT")
    Vb = persist.tile([128, TOK // 128, 130], MMDT, tag="Vb")
    OT = [persist.tile([65, TOK], MMDT, tag=f"OT{h}", name=f"OT{h}")
          for h in range(2)]

    # ones columns of V_ext (positions 64 and 129 of each 130-stripe)
    nc.scalar.copy(
        Vb[:].rearrange("p t (g x) -> p t g x", g=2)[:, :, :, 64:65],
        ones_f32[:].rearrange("p (t g x) -> p t g x", t=TOK // 128, g=2))

    # ---- phases 1-2: QKV projections + V transpose (scoped pools) ----
    with tc.tile_pool(name="xin", bufs=2) as xpool, \
         tc.tile_pool(name="vtp", bufs=1) as vtpool:
        VT = vtpool.tile([128, TOK], F32, tag="VT")
        xT_r = xT.rearrange("(c p) n -> p c n", p=128)
        for t in range(TOK // NQ):
            xt = xpool.tile([128, DCH, NQ], MMDT, tag="x")
            nc.sync.dma_start(xt[:], xT_r[:, :, t * NQ:(t + 1) * NQ])
            for name, dst in (("q", QT), ("k", KT), ("v", VT)):
                ps = opool.tile([128, NQ], F32, tag="o", name="ps_proj")
                for d in range(DCH):
                    nc.tensor.matmul(ps[:], w_sb[name][:, d, :], xt[:, d, :],
                                     start=(d == 0), stop=(d == DCH - 1))
                nc.vector.tensor_scalar_add(
                    dst[:, t * NQ:(t + 1) * NQ], ps[:], b_sb[name][:])

        # V transpose into (tok, hd) chunks
        for u in range(TOK // 128):
            pt = opool.tile([128, 128], F32, tag="o", name="pt_tr")
            nc.tensor.transpose(pt[:], VT[:, u * 128:(u + 1) * 128], ident[:])
            nc.scalar.copy(
                Vb[:, u, :].rearrange("p (g x) -> p g x", g=2)[:, :, 0:64],
                pt[:].rearrange("p (g x) -> p g x", g=2))

    # ---- phase 3: attention ----
    bpool = ctx.enter_context(tc.tile_pool(name="bin", bufs=10))
    ppool = ctx.enter_context(tc.tile_pool(name="pexp", bufs=4))
    small = ctx.enter_context(tc.tile_pool(name="small", bufs=2))
    rec = [small.tile([65, TOK], F32, tag="rec", name=f"rec{h}")
           for h in range(2)]
    B_r = [Bh[h].rearrange("(j p) q -> p j q", p=128) for h in range(2)]
    for qc in range(NQC):
        # fetch the whole bias window once; both batches reuse it
        stripes = [[bpool.tile([128, 4 * NQ], BF16, tag="b",
                               name=f"bstr{jq}{hh}") for hh in range(2)]
                   for jq in range(4)]
        for jq in range(4):
            for hh in range(2):
                nc.sync.dma_start(
                    stripes[jq][hh][:].rearrange("p (j q) -> p j q", j=4),
                    B_r[hh][:, jq * 4:(jq + 1) * 4, qc * NQ:(qc + 1) * NQ])
        for b in range(2):
            # per-batch O accumulators: b=0's drain overlaps b=1's compute
            O_ps = [opool.tile([65, NQ], F32, tag="o", name=f"O_ps{hh}")
                    for hh in range(2)]
            q0 = b * S + qc * NQ
            for jq in range(4):  # quarter-stripes of 4 k-chunks
                bstr = stripes[jq]
                for ji in range(4):
                    j = jq * 4 + ji
                    k0 = b * S + j * 128
                    # head-packed scores: h0 in PE rows 0-63, h1 in rows
                    # 64-127, issued adjacently
                    sps = spool.tile([128, 2 * NQ], F32, tag="s")
                    for hh in range(2):
                        nc.tensor.matmul(
                            sps[:, hh * NQ:(hh + 1) * NQ],
                            KT[64 * hh:64 * hh + 64, k0:k0 + 128],
                            QT[64 * hh:64 * hh + 64, q0:q0 + NQ],
                            start=True, stop=True)
                    # lag bias on DVE (PSUM in-place)
                    for hh in range(2):
                        nc.vector.tensor_add(
                            sps[:, hh * NQ:(hh + 1) * NQ],
                            sps[:, hh * NQ:(hh + 1) * NQ],
                            bstr[hh][:, ji * NQ:(ji + 1) * NQ])
                    pe = ppool.tile([128, 2 * NQ], MMDT, tag="p")
                    nc.scalar.activation(pe[:], sps[:], AF.Exp)
                    for hh in range(2):
                        nc.tensor.matmul(
                            O_ps[hh][:],
                            Vb[:, b * NJ + j, 65 * hh:65 * hh + 65],
                            pe[:, hh * NQ:(hh + 1) * NQ],
                            start=(j == 0), stop=(j == NJ - 1))
            for hh in range(2):
                # stash unnormalized O^T + denominator row; normalize later
                nc.scalar.copy(OT[hh][:, q0:q0 + NQ], O_ps[hh][:])
                nc.vector.reciprocal(rec[hh][64:65, q0:q0 + NQ],
                                     OT[hh][64:65, q0:q0 + NQ].bitcast(F32))

    # ---- phase 3b/4: normalize + output projection, pipelined ----
    for c in range(TOK // NQ):
        sl = slice(c * NQ, (c + 1) * NQ)
        for h in range(2):
            R_ps = opool.tile([64, NQ], F32, tag="o", name="R_ps")
            nc.tensor.matmul(R_ps[:], ones64[64:65, :], rec[h][64:65, sl],
                             start=True, stop=True)
            nc.vector.tensor_mul(OT[h][0:64, sl], OT[h][0:64, sl], R_ps[:])
        for u in range(4 * c, 4 * c + 4):
            ps = spool.tile([128, 2 * NQ], F32, tag="s")
            for half in range(2):
                osl = slice(half * NQ, (half + 1) * NQ)
                nc.tensor.matmul(ps[:, osl],
                                 OT[0][0:64, u * 128:(u + 1) * 128],
                                 wo0[:, osl], start=True, stop=False)
                nc.tensor.matmul(ps[:, osl],
                                 OT[1][0:64, u * 128:(u + 1) * 128],
                                 wo1[:, osl], start=False, stop=True)
            osb = ppool.tile([128, 2 * NQ], F32, tag="osb")
            nc.scalar.copy(osb[:], ps[:])
            nc.sync.dma_start(out[u * 128:(u + 1) * 128, :], osb[:])


def build_program():
    nc = bacc.Bacc("TRN2", target_bir_lowering=False, debug=False,
                   enable_asserts=False, num_devices=N_CORES)
    aps = {}
    specs = [
        ("xT", (D, TOK), MMDT), ("wq", (D, 128), MMDT), ("wk", (D, 128), MMDT),
        ("wv", (D, 128), MMDT), ("bq", (128, 1), F32), ("bk", (128, 1), F32),
        ("bv", (128, 1), F32), ("wo", (128, D), MMDT), ("B0", (S, S), BF16),
        ("B1", (S, S), BF16),
    ]
    for name, shape, dt in specs:
        aps[name] = nc.dram_tensor(name, shape, dt, kind="ExternalInput").ap()
    aps["out"] = nc.dram_tensor("out", (TOK, D), F32,
                                kind="ExternalOutput").ap()
    with tile.TileContext(nc) as tc:
        with ExitStack() as ctx:
            _body(ctx, tc, aps)
    nc.compile()
    return nc


def _get_nc():
    global _CACHED_NC
    if _CACHED_NC is None:
        _CACHED_NC = build_program()
    return _CACHED_NC


def _host_prep(x, lag, wq, bq, wk, bk, wv, bv, wo, bo, lag_bias):
    x = np.ascontiguousarray(np.asarray(x, dtype=np.float32))
    lag = np.asarray(lag).astype(np.int64)
    xT = np.ascontiguousarray(x.reshape(TOK, D).T)
    ld = np.abs(lag[:, None] - lag[None, :]).astype(np.int64)
    lag_bias = np.asarray(lag_bias, dtype=np.float32)
    scale = np.float32(1.0 / np.sqrt(DK))
    wq = np.asarray(wq, dtype=np.float32) * scale
    bq = np.asarray(bq, dtype=np.float32) * scale
    in_maps = []
    for c in range(N_CORES):
        sl = slice(c * 128, (c + 1) * 128)
        in_maps.append({
            "xT": xT,
            "wq": np.ascontiguousarray(wq[:, sl]),
            "wk": np.ascontiguousarray(np.asarray(wk, np.float32)[:, sl]),
            "wv": np.ascontiguousarray(np.asarray(wv, np.float32)[:, sl]),
            "bq": np.ascontiguousarray(bq[sl].reshape(128, 1)),
            "bk": np.ascontiguousarray(np.asarray(bk, np.float32)[sl].reshape(128, 1)),
            "bv": np.ascontiguousarray(np.asarray(bv, np.float32)[sl].reshape(128, 1)),
            "wo": np.ascontiguousarray(np.asarray(wo, np.float32)[sl, :]),
            "B0": np.ascontiguousarray(
                lag_bias[2 * c][ld].astype(ml_dtypes.bfloat16)),
            "B1": np.ascontiguousarray(
                lag_bias[2 * c + 1][ld].astype(ml_dtypes.bfloat16)),
        })
    return in_maps


def kernel(x, lag, wq, bq, wk, bk, wv, bv, wo, bo, lag_bias):
    nc = _get_nc()
    in_maps = _host_prep(x, lag, wq, bq, wk, bk, wv, bv, wo, bo, lag_bias)
    kwargs = {}
    if TRACE:
        kwargs = dict(trace=True, tmpdir=TRACE_DIR)
    res = run_bass_kernel_spmd(nc, in_maps, core_ids=list(range(N_CORES)),
                               **kwargs)
    if TRACE:
        print(f"HW exec time: {res.exec_time_ns} ns")
    total = res.results[0]["out"].astype(np.float32)
    for c in range(1, N_CORES):
        total += res.results[c]["out"]
    total += np.asarray(bo, dtype=np.float32)[None, :]
    return total.reshape(B, S, D)


# revision 24
# speedup vs baseline: 1.0071x; 1.0071x over previous
"""MultiHeadSelfAttentionWithLagBias on 8 TRN2 NeuronCores.

Sharding: tensor-parallel over heads — 16 heads / 8 cores = 2 heads per
core. Each core computes QKV projections for its head slice (full x),
attention with the lag bias for its 2 heads over both batch elements,
and a partial output projection (its 128 rows of wo). Host sums the 8
partials and adds bo.

Device layout (per core):
  xT      (1024, 4096)  x transposed, tok = b*2048 + s
  QT/KT   (128, 4096)   q^T/k^T, partitions = [h0 dk(64) | h1 dk(64)]
  V       (128, 32, 130) per 128-tok chunk: [V_h0(64) | 1 | V_h1(64) | 1]
  scores  computed transposed: S^T (k on partitions, q on free) so the
          softmax denominator falls out of the PV matmul via the ones
          column, and O^T is produced in exactly the layout the output
          projection needs as its stationary operand.
  bias    B_h (2048, 2048) host-precomputed (symmetric), DMA-streamed.
"""

import ml_dtypes
import numpy as np
from contextlib import ExitStack

import concourse.bass as bass
import concourse.bacc as bacc
import concourse.mybir as mybir
import concourse.tile as tile
from concourse.bass_utils import run_bass_kernel_spmd
from concourse.masks import make_identity

F32 = mybir.dt.float32
F32R = mybir.dt.float32r
BF16 = mybir.dt.bfloat16
AF = mybir.ActivationFunctionType

# float32r: single-pass fp32 matmul mode (1 cycle/row vs 4 for fp32),
# ~1.2e-4 relative rounding on HW. Toggle for a full-precision fallback.
USE_F32R = True
MMDT = F32R if USE_F32R else F32

N_CORES = 8
B, S, D = 2, 2048, 1024
H, DK = 16, 64
TOK = B * S              # 4096
NQ = 512                 # q-chunk (matmul free dim)
NQC = S // NQ            # 4 q-chunks per batch
NJ = S // 128            # 16 k-chunks per batch
DCH = D // 128           # 8 contraction chunks

# Set by test.py for profiling; harness leaves these untouched.
TRACE = False
TRACE_DIR = None

_CACHED_NC = None


def _body(ctx: ExitStack, tc, aps):
    nc = tc.nc
    xT, wq, wk, wv, bq, bk, bv, wo, B0, B1, out = (
        aps["xT"], aps["wq"], aps["wk"], aps["wv"], aps["bq"], aps["bk"],
        aps["bv"], aps["wo"], aps["B0"], aps["B1"], aps["out"])
    Bh = [B0, B1]

    const = ctx.enter_context(tc.tile_pool(name="const", bufs=1))
    persist = ctx.enter_context(tc.tile_pool(name="persist", bufs=1))
    spool = ctx.enter_context(tc.tile_pool(name="spsum", bufs=2, space="PSUM"))
    opool = ctx.enter_context(tc.tile_pool(name="opsum", bufs=4, space="PSUM"))

    # ---- constants ----
    ident = const.tile([128, 128], F32, tag="id")
    make_identity(nc, ident[:])
    w_sb = {}
    for name, ap in (("q", wq), ("k", wk), ("v", wv)):
        t = const.tile([128, DCH, 128], MMDT, tag=f"w{name}")
        nc.sync.dma_start(t[:], ap.rearrange("(c p) m -> p c m", p=128))
        w_sb[name] = t
    b_sb = {}
    for name, ap in (("q", bq), ("k", bk), ("v", bv)):
        t = const.tile([128, 1], F32, tag=f"b{name}")
        nc.sync.dma_start(t[:], ap[:])
        b_sb[name] = t
    # wo split into the two 64-row halves so both out-proj matmuls run at
    # partition base 0.
    wo0 = const.tile([64, D], MMDT, tag="wo0")
    wo1 = const.tile([64, D], MMDT, tag="wo1")
    nc.sync.dma_start(wo0[:], wo[0:64, :])
    nc.sync.dma_start(wo1[:], wo[64:128, :])
    # ones row at partition 64 for broadcasting the softmax reciprocal
    # (gpsimd partition_broadcast is broken on HW via this exec path)
    ones64 = const.tile([65, 64], F32, tag="ones64")
    nc.vector.memset(ones64[:], 1.0)
    # fp32 ones staged for the f32r ones-columns of Vb (memset can't
    # write f32r; ACT copy is a rounding producer)
    ones_f32 = const.tile([128, 64], F32, tag="ones_f32")
    nc.vector.memset(ones_f32[:], 1.0)

    # ---- persistent activations ----
    QT = persist.tile([128, TOK], MMDT, tag="QT")
    KT = persist.tile([128, TOK], MMDT, tag="KT")
    Vb = persist.tile([128, TOK // 128, 130], MMDT, tag="Vb")
    OT = [persist.tile([65, TOK], MMDT, tag=f"OT{h}", name=f"OT{h}")
          for h in range(2)]

    # ones columns of V_ext (positions 64 and 129 of each 130-stripe)
    nc.scalar.copy(
        Vb[:].rearrange("p t (g x) -> p t g x", g=2)[:, :, :, 64:65],
        ones_f32[:].rearrange("p (t g x) -> p t g x", t=TOK // 128, g=2))

    # ---- phases 1-2: QKV projections + V transpose (scoped pools) ----
    with tc.tile_pool(name="xin", bufs=2) as xpool, \
         tc.tile_pool(name="vtp", bufs=1) as vtpool:
        VT = vtpool.tile([128, TOK], F32, tag="VT")
        xT_r = xT.rearrange("(c p) n -> p c n", p=128)
        for t in range(TOK // NQ):
            xt = xpool.tile([128, DCH, NQ], MMDT, tag="x")
            nc.sync.dma_start(xt[:], xT_r[:, :, t * NQ:(t + 1) * NQ])
            for name, dst in (("q", QT), ("k", KT), ("v", VT)):
                ps = opool.tile([128, NQ], F32, tag="o", name="ps_proj")
                for d in range(DCH):
                    nc.tensor.matmul(ps[:], w_sb[name][:, d, :], xt[:, d, :],
                                     start=(d == 0), stop=(d == DCH - 1))
                nc.vector.tensor_scalar_add(
                    dst[:, t * NQ:(t + 1) * NQ], ps[:], b_sb[name][:])

        # V transpose into (tok, hd) chunks
        for u in range(TOK // 128):
            pt = opool.tile([128, 128], F32, tag="o", name="pt_tr")
            nc.tensor.transpose(pt[:], VT[:, u * 128:(u + 1) * 128], ident[:])
            nc.scalar.copy(
                Vb[:, u, :].rearrange("p (g x) -> p g x", g=2)[:, :, 0:64],
                pt[:].rearrange("p (g x) -> p g x", g=2))

    # ---- phase 3: attention ----
    bpool = ctx.enter_context(tc.tile_pool(name="bin", bufs=4))
    ppool = ctx.enter_context(tc.tile_pool(name="pexp", bufs=4))
    small = ctx.enter_context(tc.tile_pool(name="small", bufs=2))
    rec = [small.tile([65, TOK], F32, tag="rec", name=f"rec{h}")
           for h in range(2)]
    B_r = [Bh[h].rearrange("(j p) q -> p j q", p=128) for h in range(2)]
    for qc in range(NQC):
        O_ps = [[opool.tile([65, NQ], F32, tag="o", name=f"O_ps{hh}{bb}")
                 for bb in range(2)] for hh in range(2)]
        for jq in range(4):  # quarter-stripes of 4 k-chunks
            bstr = [bpool.tile([128, 4 * NQ], BF16, tag="b", name=f"bstr{hh}")
                    for hh in range(2)]
            for hh in range(2):
                nc.sync.dma_start(
                    bstr[hh][:].rearrange("p (j q) -> p j q", j=4),
                    B_r[hh][:, jq * 4:(jq + 1) * 4, qc * NQ:(qc + 1) * NQ])
            for b in range(2):
                q0 = b * S + qc * NQ
                for ji in range(4):
                    j = jq * 4 + ji
                    k0 = b * S + j * 128
                    # head-packed scores: h0 in PE rows 0-63, h1 in rows
                    # 64-127, issued adjacently
                    sps = spool.tile([128, 2 * NQ], F32, tag="s")
                    for hh in range(2):
                        nc.tensor.matmul(
                            sps[:, hh * NQ:(hh + 1) * NQ],
                            KT[64 * hh:64 * hh + 64, k0:k0 + 128],
                            QT[64 * hh:64 * hh + 64, q0:q0 + NQ],
                            start=True, stop=True)
                    # lag bias on DVE (PSUM in-place)
                    for hh in range(2):
                        nc.vector.tensor_add(
                            sps[:, hh * NQ:(hh + 1) * NQ],
                            sps[:, hh * NQ:(hh + 1) * NQ],
                            bstr[hh][:, ji * NQ:(ji + 1) * NQ])
                    pe = ppool.tile([128, 2 * NQ], MMDT, tag="p")
                    nc.scalar.activation(pe[:], sps[:], AF.Exp)
                    for hh in range(2):
                        nc.tensor.matmul(
                            O_ps[hh][b][:],
                            Vb[:, b * NJ + j, 65 * hh:65 * hh + 65],
                            pe[:, hh * NQ:(hh + 1) * NQ],
                            start=(j == 0), stop=(j == NJ - 1))
        for hh in range(2):
            for b in range(2):
                q0 = b * S + qc * NQ
                # stash unnormalized O^T + denominator row; normalize later
                nc.scalar.copy(OT[hh][:, q0:q0 + NQ], O_ps[hh][b][:])
                nc.vector.reciprocal(rec[hh][64:65, q0:q0 + NQ],
                                     OT[hh][64:65, q0:q0 + NQ].bitcast(F32))

    # ---- phase 3b/4: normalize + output projection, pipelined ----
    for c in range(TOK // NQ):
        sl = slice(c * NQ, (c + 1) * NQ)
        for h in range(2):
            R_ps = opool.tile([64, NQ], F32, tag="o", name="R_ps")
            nc.tensor.matmul(R_ps[:], ones64[64:65, :], rec[h][64:65, sl],
                             start=True, stop=True)
            nc.vector.tensor_mul(OT[h][0:64, sl], OT[h][0:64, sl], R_ps[:])
        for u in range(4 * c, 4 * c + 4):
            ps = spool.tile([128, 2 * NQ], F32, tag="s")
            for half in range(2):
                osl = slice(half * NQ, (half + 1) * NQ)
                nc.tensor.matmul(ps[:, osl],
                                 OT[0][0:64, u * 128:(u + 1) * 128],
                                 wo0[:, osl], start=True, stop=False)
                nc.tensor.matmul(ps[:, osl],
                                 OT[1][0:64, u * 128:(u + 1) * 128],
                                 wo1[:, osl], start=False, stop=True)
            osb = ppool.tile([128, 2 * NQ], F32, tag="osb")
            nc.scalar.copy(osb[:], ps[:])
            nc.sync.dma_start(out[u * 128:(u + 1) * 128, :], osb[:])


def build_program():
    nc = bacc.Bacc("TRN2", target_bir_lowering=False, debug=False,
                   enable_asserts=False, num_devices=N_CORES)
    aps = {}
    specs = [
        ("xT", (D, TOK), MMDT), ("wq", (D, 128), MMDT), ("wk", (D, 128), MMDT),
        ("wv", (D, 128), MMDT), ("bq", (128, 1), F32), ("bk", (128, 1), F32),
        ("bv", (128, 1), F32), ("wo", (128, D), MMDT), ("B0", (S, S), BF16),
        ("B1", (S, S), BF16),
    ]
    for name, shape, dt in specs:
        aps[name] = nc.dram_tensor(name, shape, dt, kind="ExternalInput").ap()
    aps["out"] = nc.dram_tensor("out", (TOK, D), F32,
                                kind="ExternalOutput").ap()
    with tile.TileContext(nc) as tc:
        with ExitStack() as ctx:
            _body(ctx, tc, aps)
    nc.compile()
    return nc


def _get_nc():
    global _CACHED_NC
    if _CACHED_NC is None:
        _CACHED_NC = build_program()
    return _CACHED_NC


def _host_prep(x, lag, wq, bq, wk, bk, wv, bv, wo, bo, lag_bias):
    x = np.ascontiguousarray(np.asarray(x, dtype=np.float32))
    lag = np.asarray(lag).astype(np.int64)
    xT = np.ascontiguousarray(x.reshape(TOK, D).T)
    ld = np.abs(lag[:, None] - lag[None, :]).astype(np.int64)
    lag_bias = np.asarray(lag_bias, dtype=np.float32)
    scale = np.float32(1.0 / np.sqrt(DK))
    wq = np.asarray(wq, dtype=np.float32) * scale
    bq = np.asarray(bq, dtype=np.float32) * scale
    in_maps = []
    for c in range(N_CORES):
        sl = slice(c * 128, (c + 1) * 128)
        in_maps.append({
            "xT": xT,
            "wq": np.ascontiguousarray(wq[:, sl]),
            "wk": np.ascontiguousarray(np.asarray(wk, np.float32)[:, sl]),
            "wv": np.ascontiguousarray(np.asarray(wv, np.float32)[:, sl]),
            "bq": np.ascontiguousarray(bq[sl].reshape(128, 1)),
            "bk": np.ascontiguousarray(np.asarray(bk, np.float32)[sl].reshape(128, 1)),
            "bv": np.ascontiguousarray(np.asarray(bv, np.float32)[sl].reshape(128, 1)),
            "wo": np.ascontiguousarray(np.asarray(wo, np.float32)[sl, :]),
            "B0": np.ascontiguousarray(
                lag_bias[2 * c][ld].astype(ml_dtypes.bfloat16)),
            "B1": np.ascontiguousarray(
                lag_bias[2 * c + 1][ld].astype(ml_dtypes.bfloat16)),
        })
    return in_maps


def kernel(x, lag, wq, bq, wk, bk, wv, bv, wo, bo, lag_bias):
    nc = _get_nc()
    in_maps = _host_prep(x, lag, wq, bq, wk, bk, wv, bv, wo, bo, lag_bias)
    kwargs = {}
    if TRACE:
        kwargs = dict(trace=True, tmpdir=TRACE_DIR)
    res = run_bass_kernel_spmd(nc, in_maps, core_ids=list(range(N_CORES)),
                               **kwargs)
    if TRACE:
        print(f"HW exec time: {res.exec_time_ns} ns")
    total = res.results[0]["out"].astype(np.float32)
    for c in range(1, N_CORES):
        total += res.results[c]["out"]
    total += np.asarray(bo, dtype=np.float32)[None, :]
    return total.reshape(B, S, D)


# revision 25
# speedup vs baseline: 1.0135x; 1.0064x over previous
"""MultiHeadSelfAttentionWithLagBias on 8 TRN2 NeuronCores.

Sharding: tensor-parallel over heads — 16 heads / 8 cores = 2 heads per
core. Each core computes QKV projections for its head slice (full x),
attention with the lag bias for its 2 heads over both batch elements,
and a partial output projection (its 128 rows of wo). Host sums the 8
partials and adds bo.

Device layout (per core):
  xT      (1024, 4096)  x transposed, tok = b*2048 + s
  QT/KT   (128, 4096)   q^T/k^T, partitions = [h0 dk(64) | h1 dk(64)]
  V       (128, 32, 130) per 128-tok chunk: [V_h0(64) | 1 | V_h1(64) | 1]
  scores  computed transposed: S^T (k on partitions, q on free) so the
          softmax denominator falls out of the PV matmul via the ones
          column, and O^T is produced in exactly the layout the output
          projection needs as its stationary operand.
  bias    B_h (2048, 2048) host-precomputed (symmetric), DMA-streamed.
"""

import ml_dtypes
import numpy as np
from contextlib import ExitStack

import concourse.bass as bass
import concourse.bacc as bacc
import concourse.mybir as mybir
import concourse.tile as tile
from concourse.bass_utils import run_bass_kernel_spmd
from concourse.masks import make_identity

F32 = mybir.dt.float32
F32R = mybir.dt.float32r
BF16 = mybir.dt.bfloat16
AF = mybir.ActivationFunctionType

# float32r: single-pass fp32 matmul mode (1 cycle/row vs 4 for fp32),
# ~1.2e-4 relative rounding on HW. Toggle for a full-precision fallback.
USE_F32R = True
MMDT = F32R if USE_F32R else F32

N_CORES = 8
B, S, D = 2, 2048, 1024
H, DK = 16, 64
TOK = B * S              # 4096
NQ = 512                 # q-chunk (matmul free dim)
NQC = S // NQ            # 4 q-chunks per batch
NJ = S // 128            # 16 k-chunks per batch
DCH = D // 128           # 8 contraction chunks

# Set by test.py for profiling; harness leaves these untouched.
TRACE = False
TRACE_DIR = None

_CACHED_NC = None


def _body(ctx: ExitStack, tc, aps):
    nc = tc.nc
    xT, wq, wk, wv, bq, bk, bv, wo, B0, B1, out = (
        aps["xT"], aps["wq"], aps["wk"], aps["wv"], aps["bq"], aps["bk"],
        aps["bv"], aps["wo"], aps["B0"], aps["B1"], aps["out"])
    Bh = [B0, B1]

    const = ctx.enter_context(tc.tile_pool(name="const", bufs=1))
    persist = ctx.enter_context(tc.tile_pool(name="persist", bufs=1))
    spool = ctx.enter_context(tc.tile_pool(name="spsum", bufs=2, space="PSUM"))
    opool = ctx.enter_context(tc.tile_pool(name="opsum", bufs=4, space="PSUM"))

    # ---- constants ----
    ident = const.tile([128, 128], F32, tag="id")
    make_identity(nc, ident[:])
    w_sb = {}
    for name, ap in (("q", wq), ("k", wk), ("v", wv)):
        t = const.tile([128, DCH, 128], MMDT, tag=f"w{name}")
        nc.sync.dma_start(t[:], ap.rearrange("(c p) m -> p c m", p=128))
        w_sb[name] = t
    b_sb = {}
    for name, ap in (("q", bq), ("k", bk), ("v", bv)):
        t = const.tile([128, 1], F32, tag=f"b{name}")
        nc.sync.dma_start(t[:], ap[:])
        b_sb[name] = t
    # wo split into the two 64-row halves so both out-proj matmuls run at
    # partition base 0.
    wo0 = const.tile([64, D], MMDT, tag="wo0")
    wo1 = const.tile([64, D], MMDT, tag="wo1")
    nc.sync.dma_start(wo0[:], wo[0:64, :])
    nc.sync.dma_start(wo1[:], wo[64:128, :])
    # ones row at partition 64 for broadcasting the softmax reciprocal
    # (gpsimd partition_broadcast is broken on HW via this exec path)
    ones64 = const.tile([65, 64], F32, tag="ones64")
    nc.vector.memset(ones64[:], 1.0)
    # fp32 ones staged for the f32r ones-columns of Vb (memset can't
    # write f32r; ACT copy is a rounding producer)
    ones_f32 = const.tile([128, 64], F32, tag="ones_f32")
    nc.vector.memset(ones_f32[:], 1.0)

    # ---- persistent activations ----
    QT = persist.tile([128, TOK], MMDT, tag="QT")
    KT = persist.tile([128, TOK], MMDT, tag="KT")
    Vb = persist.tile([128, TOK // 128, 130], MMDT, tag="Vb")
    OT = [persist.tile([65, TOK], MMDT, tag=f"OT{h}", name=f"OT{h}")
          for h in range(2)]

    # ones columns of V_ext (positions 64 and 129 of each 130-stripe)
    nc.scalar.copy(
        Vb[:].rearrange("p t (g x) -> p t g x", g=2)[:, :, :, 64:65],
        ones_f32[:].rearrange("p (t g x) -> p t g x", t=TOK // 128, g=2))

    # ---- phases 1-2: QKV projections + V transpose (scoped pools) ----
    with tc.tile_pool(name="xin", bufs=2) as xpool, \
         tc.tile_pool(name="vtp", bufs=1) as vtpool:
        VT = vtpool.tile([128, TOK], F32, tag="VT")
        xT_r = xT.rearrange("(c p) n -> p c n", p=128)
        for t in range(TOK // NQ):
            xt = xpool.tile([128, DCH, NQ], MMDT, tag="x")
            nc.sync.dma_start(xt[:], xT_r[:, :, t * NQ:(t + 1) * NQ])
            for name, dst in (("q", QT), ("k", KT), ("v", VT)):
                ps = opool.tile([128, NQ], F32, tag="o", name="ps_proj")
                for d in range(DCH):
                    nc.tensor.matmul(ps[:], w_sb[name][:, d, :], xt[:, d, :],
                                     start=(d == 0), stop=(d == DCH - 1))
                nc.vector.tensor_scalar_add(
                    dst[:, t * NQ:(t + 1) * NQ], ps[:], b_sb[name][:])

        # V transpose into (tok, hd) chunks
        for u in range(TOK // 128):
            pt = opool.tile([128, 128], F32, tag="o", name="pt_tr")
            nc.tensor.transpose(pt[:], VT[:, u * 128:(u + 1) * 128], ident[:])
            nc.scalar.copy(
                Vb[:, u, :].rearrange("p (g x) -> p g x", g=2)[:, :, 0:64],
                pt[:].rearrange("p (g x) -> p g x", g=2))

    # ---- phase 3: attention ----
    bpool = ctx.enter_context(tc.tile_pool(name="bin", bufs=6))
    ppool = ctx.enter_context(tc.tile_pool(name="pexp", bufs=6))
    small = ctx.enter_context(tc.tile_pool(name="small", bufs=2))
    rec = [small.tile([65, TOK], F32, tag="rec", name=f"rec{h}")
           for h in range(2)]
    B_r = [Bh[h].rearrange("(j p) q -> p j q", p=128) for h in range(2)]
    for qc in range(NQC):
        O_ps = [[opool.tile([65, NQ], F32, tag="o", name=f"O_ps{hh}{bb}")
                 for bb in range(2)] for hh in range(2)]
        for jq in range(4):  # quarter-stripes of 4 k-chunks
            bstr = [bpool.tile([128, 4 * NQ], BF16, tag="b", name=f"bstr{hh}")
                    for hh in range(2)]
            for hh in range(2):
                nc.sync.dma_start(
                    bstr[hh][:].rearrange("p (j q) -> p j q", j=4),
                    B_r[hh][:, jq * 4:(jq + 1) * 4, qc * NQ:(qc + 1) * NQ])
            for b in range(2):
                q0 = b * S + qc * NQ
                for ji in range(4):
                    j = jq * 4 + ji
                    k0 = b * S + j * 128
                    # head-packed scores: h0 in PE rows 0-63, h1 in rows
                    # 64-127, issued adjacently
                    sps = spool.tile([128, 2 * NQ], F32, tag="s")
                    for hh in range(2):
                        nc.tensor.matmul(
                            sps[:, hh * NQ:(hh + 1) * NQ],
                            KT[64 * hh:64 * hh + 64, k0:k0 + 128],
                            QT[64 * hh:64 * hh + 64, q0:q0 + NQ],
                            start=True, stop=True)
                    # lag bias on DVE (PSUM in-place)
                    for hh in range(2):
                        nc.vector.tensor_add(
                            sps[:, hh * NQ:(hh + 1) * NQ],
                            sps[:, hh * NQ:(hh + 1) * NQ],
                            bstr[hh][:, ji * NQ:(ji + 1) * NQ])
                    pe = ppool.tile([128, 2 * NQ], MMDT, tag="p")
                    nc.scalar.activation(pe[:], sps[:], AF.Exp)
                    for hh in range(2):
                        nc.tensor.matmul(
                            O_ps[hh][b][:],
                            Vb[:, b * NJ + j, 65 * hh:65 * hh + 65],
                            pe[:, hh * NQ:(hh + 1) * NQ],
                            start=(j == 0), stop=(j == NJ - 1))
        for hh in range(2):
            for b in range(2):
                q0 = b * S + qc * NQ
                # stash unnormalized O^T + denominator row; normalize later
                nc.scalar.copy(OT[hh][:, q0:q0 + NQ], O_ps[hh][b][:])
                nc.vector.reciprocal(rec[hh][64:65, q0:q0 + NQ],
                                     OT[hh][64:65, q0:q0 + NQ].bitcast(F32))

    # ---- phase 3b/4: normalize + output projection, pipelined ----
    for c in range(TOK // NQ):
        sl = slice(c * NQ, (c + 1) * NQ)
        for h in range(2):
            R_ps = opool.tile([64, NQ], F32, tag="o", name="R_ps")
            nc.tensor.matmul(R_ps[:], ones64[64:65, :], rec[h][64:65, sl],
                             start=True, stop=True)
            nc.vector.tensor_mul(OT[h][0:64, sl], OT[h][0:64, sl], R_ps[:])
        for u in range(4 * c, 4 * c + 4):
            ps = spool.tile([128, 2 * NQ], F32, tag="s")
            for half in range(2):
                osl = slice(half * NQ, (half + 1) * NQ)
                nc.tensor.matmul(ps[:, osl],
                                 OT[0][0:64, u * 128:(u + 1) * 128],
                                 wo0[:, osl], start=True, stop=False)
                nc.tensor.matmul(ps[:, osl],
                                 OT[1][0:64, u * 128:(u + 1) * 128],
                                 wo1[:, osl], start=False, stop=True)
            osb = ppool.tile([128, 2 * NQ], F32, tag="osb")
            nc.scalar.copy(osb[:], ps[:])
            nc.sync.dma_start(out[u * 128:(u + 1) * 128, :], osb[:])


def build_program():
    nc = bacc.Bacc("TRN2", target_bir_lowering=False, debug=False,
                   enable_asserts=False, num_devices=N_CORES)
    aps = {}
    specs = [
        ("xT", (D, TOK), MMDT), ("wq", (D, 128), MMDT), ("wk", (D, 128), MMDT),
        ("wv", (D, 128), MMDT), ("bq", (128, 1), F32), ("bk", (128, 1), F32),
        ("bv", (128, 1), F32), ("wo", (128, D), MMDT), ("B0", (S, S), BF16),
        ("B1", (S, S), BF16),
    ]
    for name, shape, dt in specs:
        aps[name] = nc.dram_tensor(name, shape, dt, kind="ExternalInput").ap()
    aps["out"] = nc.dram_tensor("out", (TOK, D), F32,
                                kind="ExternalOutput").ap()
    with tile.TileContext(nc) as tc:
        with ExitStack() as ctx:
            _body(ctx, tc, aps)
    nc.compile()
    return nc


def _get_nc():
    global _CACHED_NC
    if _CACHED_NC is None:
        _CACHED_NC = build_program()
    return _CACHED_NC


def _host_prep(x, lag, wq, bq, wk, bk, wv, bv, wo, bo, lag_bias):
    x = np.ascontiguousarray(np.asarray(x, dtype=np.float32))
    lag = np.asarray(lag).astype(np.int64)
    xT = np.ascontiguousarray(x.reshape(TOK, D).T)
    ld = np.abs(lag[:, None] - lag[None, :]).astype(np.int64)
    lag_bias = np.asarray(lag_bias, dtype=np.float32)
    scale = np.float32(1.0 / np.sqrt(DK))
    wq = np.asarray(wq, dtype=np.float32) * scale
    bq = np.asarray(bq, dtype=np.float32) * scale
    in_maps = []
    for c in range(N_CORES):
        sl = slice(c * 128, (c + 1) * 128)
        in_maps.append({
            "xT": xT,
            "wq": np.ascontiguousarray(wq[:, sl]),
            "wk": np.ascontiguousarray(np.asarray(wk, np.float32)[:, sl]),
            "wv": np.ascontiguousarray(np.asarray(wv, np.float32)[:, sl]),
            "bq": np.ascontiguousarray(bq[sl].reshape(128, 1)),
            "bk": np.ascontiguousarray(np.asarray(bk, np.float32)[sl].reshape(128, 1)),
            "bv": np.ascontiguousarray(np.asarray(bv, np.float32)[sl].reshape(128, 1)),
            "wo": np.ascontiguousarray(np.asarray(wo, np.float32)[sl, :]),
            "B0": np.ascontiguousarray(
                lag_bias[2 * c][ld].astype(ml_dtypes.bfloat16)),
            "B1": np.ascontiguousarray(
                lag_bias[2 * c + 1][ld].astype(ml_dtypes.bfloat16)),
        })
    return in_maps


def kernel(x, lag, wq, bq, wk, bk, wv, bv, wo, bo, lag_bias):
    nc = _get_nc()
    in_maps = _host_prep(x, lag, wq, bq, wk, bk, wv, bv, wo, bo, lag_bias)
    kwargs = {}
    if TRACE:
        kwargs = dict(trace=True, tmpdir=TRACE_DIR)
    res = run_bass_kernel_spmd(nc, in_maps, core_ids=list(range(N_CORES)),
                               **kwargs)
    if TRACE:
        print(f"HW exec time: {res.exec_time_ns} ns")
    total = res.results[0]["out"].astype(np.float32)
    for c in range(1, N_CORES):
        total += res.results[c]["out"]
    total += np.asarray(bo, dtype=np.float32)[None, :]
    return total.reshape(B, S, D)


# revision 26
# speedup vs baseline: 1.0494x; 1.0354x over previous
"""MultiHeadSelfAttentionWithLagBias on 8 TRN2 NeuronCores.

Sharding: tensor-parallel over heads — 16 heads / 8 cores = 2 heads per
core. Each core computes QKV projections for its head slice (full x),
attention with the lag bias for its 2 heads over both batch elements,
and a partial output projection (its 128 rows of wo). Host sums the 8
partials and adds bo.

Device layout (per core):
  xT      (1024, 4096)  x transposed, tok = b*2048 + s
  QT/KT   (128, 4096)   q^T/k^T, partitions = [h0 dk(64) | h1 dk(64)]
  V       (128, 32, 130) per 128-tok chunk: [V_h0(64) | 1 | V_h1(64) | 1]
  scores  computed transposed: S^T (k on partitions, q on free) so the
          softmax denominator falls out of the PV matmul via the ones
          column, and O^T is produced in exactly the layout the output
          projection needs as its stationary operand.
  bias    B_h (2048, 2048) host-precomputed (symmetric), DMA-streamed.
"""

import ml_dtypes
import numpy as np
from contextlib import ExitStack

import concourse.bass as bass
import concourse.bacc as bacc
import concourse.mybir as mybir
import concourse.tile as tile
from concourse.bass_utils import run_bass_kernel_spmd
from concourse.masks import make_identity

F32 = mybir.dt.float32
F32R = mybir.dt.float32r
BF16 = mybir.dt.bfloat16
AF = mybir.ActivationFunctionType

# float32r: single-pass fp32 matmul mode (1 cycle/row vs 4 for fp32),
# ~1.2e-4 relative rounding on HW. Toggle for a full-precision fallback.
USE_F32R = True
MMDT = F32R if USE_F32R else F32

N_CORES = 8
B, S, D = 2, 2048, 1024
H, DK = 16, 64
TOK = B * S              # 4096
NQ = 512                 # q-chunk (matmul free dim)
NQC = S // NQ            # 4 q-chunks per batch
NJ = S // 128            # 16 k-chunks per batch
DCH = D // 128           # 8 contraction chunks

# Set by test.py for profiling; harness leaves these untouched.
TRACE = False
TRACE_DIR = None

_CACHED_NC = None


def _body(ctx: ExitStack, tc, aps):
    nc = tc.nc
    xT, wq, wk, wv, bq, bk, bv, wo, B0, B1, out = (
        aps["xT"], aps["wq"], aps["wk"], aps["wv"], aps["bq"], aps["bk"],
        aps["bv"], aps["wo"], aps["B0"], aps["B1"], aps["out"])
    Bh = [B0, B1]

    const = ctx.enter_context(tc.tile_pool(name="const", bufs=1))
    persist = ctx.enter_context(tc.tile_pool(name="persist", bufs=1))
    spool = ctx.enter_context(tc.tile_pool(name="spsum", bufs=2, space="PSUM"))
    opool = ctx.enter_context(tc.tile_pool(name="opsum", bufs=4, space="PSUM"))

    # ---- constants ----
    ident = const.tile([128, 128], F32, tag="id")
    make_identity(nc, ident[:])
    w_sb = {}
    for name, ap in (("q", wq), ("k", wk), ("v", wv)):
        t = const.tile([128, DCH, 128], MMDT, tag=f"w{name}")
        nc.sync.dma_start(t[:], ap.rearrange("(c p) m -> p c m", p=128))
        w_sb[name] = t
    b_sb = {}
    for name, ap in (("q", bq), ("k", bk), ("v", bv)):
        t = const.tile([128, 1], F32, tag=f"b{name}")
        nc.sync.dma_start(t[:], ap[:])
        b_sb[name] = t
    # wo split into the two 64-row halves so both out-proj matmuls run at
    # partition base 0.
    wo0 = const.tile([64, D], MMDT, tag="wo0")
    wo1 = const.tile([64, D], MMDT, tag="wo1")
    nc.sync.dma_start(wo0[:], wo[0:64, :])
    nc.sync.dma_start(wo1[:], wo[64:128, :])
    # ones row at partition 64 for broadcasting the softmax reciprocal
    # (gpsimd partition_broadcast is broken on HW via this exec path)
    ones64 = const.tile([65, 64], F32, tag="ones64")
    nc.vector.memset(ones64[:], 1.0)
    # fp32 ones staged for the f32r ones-columns of Vb (memset can't
    # write f32r; ACT copy is a rounding producer)
    ones_f32 = const.tile([128, 64], F32, tag="ones_f32")
    nc.vector.memset(ones_f32[:], 1.0)

    # ---- persistent activations ----
    QT = persist.tile([128, TOK], MMDT, tag="QT")
    KT = persist.tile([128, TOK], MMDT, tag="KT")
    Vb = persist.tile([128, TOK // 128, 130], MMDT, tag="Vb")
    OT = [persist.tile([65, TOK], MMDT, tag=f"OT{h}", name=f"OT{h}")
          for h in range(2)]

    # ones columns of V_ext (positions 64 and 129 of each 130-stripe)
    nc.scalar.copy(
        Vb[:].rearrange("p t (g x) -> p t g x", g=2)[:, :, :, 64:65],
        ones_f32[:].rearrange("p (t g x) -> p t g x", t=TOK // 128, g=2))

    # ---- phases 1-2: QKV projections + V transpose (scoped pools) ----
    with tc.tile_pool(name="xin", bufs=2) as xpool, \
         tc.tile_pool(name="vtp", bufs=1) as vtpool:
        VT = vtpool.tile([128, TOK], F32, tag="VT")
        xT_r = xT.rearrange("(c p) n -> p c n", p=128)
        for t in range(TOK // NQ):
            xt = xpool.tile([128, DCH, NQ], MMDT, tag="x")
            nc.sync.dma_start(xt[:], xT_r[:, :, t * NQ:(t + 1) * NQ])
            for name, dst in (("q", QT), ("k", KT), ("v", VT)):
                ps = opool.tile([128, NQ], F32, tag="o", name="ps_proj")
                for d in range(DCH):
                    nc.tensor.matmul(ps[:], w_sb[name][:, d, :], xt[:, d, :],
                                     start=(d == 0), stop=(d == DCH - 1))
                nc.vector.tensor_scalar_add(
                    dst[:, t * NQ:(t + 1) * NQ], ps[:], b_sb[name][:])

        # V transpose into (tok, hd) chunks
        for u in range(TOK // 128):
            pt = opool.tile([128, 128], F32, tag="o", name="pt_tr")
            nc.tensor.transpose(pt[:], VT[:, u * 128:(u + 1) * 128], ident[:])
            nc.scalar.copy(
                Vb[:, u, :].rearrange("p (g x) -> p g x", g=2)[:, :, 0:64],
                pt[:].rearrange("p (g x) -> p g x", g=2))

    # ---- phase 3: attention ----
    bpool = ctx.enter_context(tc.tile_pool(name="bin", bufs=3))
    ppool = ctx.enter_context(tc.tile_pool(name="pexp", bufs=6))
    small = ctx.enter_context(tc.tile_pool(name="small", bufs=2))
    rec = [small.tile([65, TOK], F32, tag="rec", name=f"rec{h}")
           for h in range(2)]
    B_r = [Bh[h].rearrange("(j p) q -> p j q", p=128) for h in range(2)]
    for qc in range(NQC):
        O_ps = [[opool.tile([65, NQ], F32, tag="o", name=f"O_ps{hh}{bb}")
                 for bb in range(2)] for hh in range(2)]
        for jq in range(4):  # quarter-stripes of 4 k-chunks
            # both heads interleaved per k-chunk -> one wide DVE add later
            bstr = bpool.tile([128, 4, 2, NQ], BF16, tag="b")
            for hh in range(2):
                nc.sync.dma_start(
                    bstr[:, :, hh, :],
                    B_r[hh][:, jq * 4:(jq + 1) * 4, qc * NQ:(qc + 1) * NQ])
            for b in range(2):
                q0 = b * S + qc * NQ
                for ji in range(4):
                    j = jq * 4 + ji
                    k0 = b * S + j * 128
                    # head-packed scores: h0 in PE rows 0-63, h1 in rows
                    # 64-127, issued adjacently
                    sps = spool.tile([128, 2 * NQ], F32, tag="s")
                    for hh in range(2):
                        nc.tensor.matmul(
                            sps[:, hh * NQ:(hh + 1) * NQ],
                            KT[64 * hh:64 * hh + 64, k0:k0 + 128],
                            QT[64 * hh:64 * hh + 64, q0:q0 + NQ],
                            start=True, stop=True)
                    # lag bias on DVE (PSUM in-place, both heads at once)
                    nc.vector.tensor_add(
                        sps[:], sps[:],
                        bstr[:, ji, :, :].rearrange("p g q -> p (g q)"))
                    pe = ppool.tile([128, 2 * NQ], MMDT, tag="p")
                    nc.scalar.activation(pe[:], sps[:], AF.Exp)
                    for hh in range(2):
                        nc.tensor.matmul(
                            O_ps[hh][b][:],
                            Vb[:, b * NJ + j, 65 * hh:65 * hh + 65],
                            pe[:, hh * NQ:(hh + 1) * NQ],
                            start=(j == 0), stop=(j == NJ - 1))
        for hh in range(2):
            for b in range(2):
                q0 = b * S + qc * NQ
                # stash unnormalized O^T + denominator row; normalize later
                nc.scalar.copy(OT[hh][:, q0:q0 + NQ], O_ps[hh][b][:])
                nc.vector.reciprocal(rec[hh][64:65, q0:q0 + NQ],
                                     OT[hh][64:65, q0:q0 + NQ].bitcast(F32))

    # ---- phase 3b/4: normalize + output projection, pipelined ----
    for c in range(TOK // NQ):
        sl = slice(c * NQ, (c + 1) * NQ)
        for h in range(2):
            R_ps = opool.tile([64, NQ], F32, tag="o", name="R_ps")
            nc.tensor.matmul(R_ps[:], ones64[64:65, :], rec[h][64:65, sl],
                             start=True, stop=True)
            nc.vector.tensor_mul(OT[h][0:64, sl], OT[h][0:64, sl], R_ps[:])
        for u in range(4 * c, 4 * c + 4):
            ps = spool.tile([128, 2 * NQ], F32, tag="s")
            for half in range(2):
                osl = slice(half * NQ, (half + 1) * NQ)
                nc.tensor.matmul(ps[:, osl],
                                 OT[0][0:64, u * 128:(u + 1) * 128],
                                 wo0[:, osl], start=True, stop=False)
                nc.tensor.matmul(ps[:, osl],
                                 OT[1][0:64, u * 128:(u + 1) * 128],
                                 wo1[:, osl], start=False, stop=True)
            osb = ppool.tile([128, 2 * NQ], F32, tag="osb")
            nc.scalar.copy(osb[:], ps[:])
            nc.sync.dma_start(out[u * 128:(u + 1) * 128, :], osb[:])


def build_program():
    nc = bacc.Bacc("TRN2", target_bir_lowering=False, debug=False,
                   enable_asserts=False, num_devices=N_CORES)
    aps = {}
    specs = [
        ("xT", (D, TOK), MMDT), ("wq", (D, 128), MMDT), ("wk", (D, 128), MMDT),
        ("wv", (D, 128), MMDT), ("bq", (128, 1), F32), ("bk", (128, 1), F32),
        ("bv", (128, 1), F32), ("wo", (128, D), MMDT), ("B0", (S, S), BF16),
        ("B1", (S, S), BF16),
    ]
    for name, shape, dt in specs:
        aps[name] = nc.dram_tensor(name, shape, dt, kind="ExternalInput").ap()
    aps["out"] = nc.dram_tensor("out", (TOK, D), F32,
                                kind="ExternalOutput").ap()
    with tile.TileContext(nc) as tc:
        with ExitStack() as ctx:
            _body(ctx, tc, aps)
    nc.compile()
    return nc


def _get_nc():
    global _CACHED_NC
    if _CACHED_NC is None:
        _CACHED_NC = build_program()
    return _CACHED_NC


def _host_prep(x, lag, wq, bq, wk, bk, wv, bv, wo, bo, lag_bias):
    x = np.ascontiguousarray(np.asarray(x, dtype=np.float32))
    lag = np.asarray(lag).astype(np.int64)
    xT = np.ascontiguousarray(x.reshape(TOK, D).T)
    ld = np.abs(lag[:, None] - lag[None, :]).astype(np.int64)
    lag_bias = np.asarray(lag_bias, dtype=np.float32)
    scale = np.float32(1.0 / np.sqrt(DK))
    wq = np.asarray(wq, dtype=np.float32) * scale
    bq = np.asarray(bq, dtype=np.float32) * scale
    in_maps = []
    for c in range(N_CORES):
        sl = slice(c * 128, (c + 1) * 128)
        in_maps.append({
            "xT": xT,
            "wq": np.ascontiguousarray(wq[:, sl]),
            "wk": np.ascontiguousarray(np.asarray(wk, np.float32)[:, sl]),
            "wv": np.ascontiguousarray(np.asarray(wv, np.float32)[:, sl]),
            "bq": np.ascontiguousarray(bq[sl].reshape(128, 1)),
            "bk": np.ascontiguousarray(np.asarray(bk, np.float32)[sl].reshape(128, 1)),
            "bv": np.ascontiguousarray(np.asarray(bv, np.float32)[sl].reshape(128, 1)),
            "wo": np.ascontiguousarray(np.asarray(wo, np.float32)[sl, :]),
            "B0": np.ascontiguousarray(
                lag_bias[2 * c][ld].astype(ml_dtypes.bfloat16)),
            "B1": np.ascontiguousarray(
                lag_bias[2 * c + 1][ld].astype(ml_dtypes.bfloat16)),
        })
    return in_maps


def kernel(x, lag, wq, bq, wk, bk, wv, bv, wo, bo, lag_bias):
    nc = _get_nc()
    in_maps = _host_prep(x, lag, wq, bq, wk, bk, wv, bv, wo, bo, lag_bias)
    kwargs = {}
    if TRACE:
        kwargs = dict(trace=True, tmpdir=TRACE_DIR)
    res = run_bass_kernel_spmd(nc, in_maps, core_ids=list(range(N_CORES)),
                               **kwargs)
    if TRACE:
        print(f"HW exec time: {res.exec_time_ns} ns")
    total = res.results[0]["out"].astype(np.float32)
    for c in range(1, N_CORES):
        total += res.results[c]["out"]
    total += np.asarray(bo, dtype=np.float32)[None, :]
    return total.reshape(B, S, D)


# revision 28
# speedup vs baseline: 1.0505x; 1.0011x over previous
"""MultiHeadSelfAttentionWithLagBias on 8 TRN2 NeuronCores.

Sharding: tensor-parallel over heads — 16 heads / 8 cores = 2 heads per
core. Each core computes QKV projections for its head slice (full x),
attention with the lag bias for its 2 heads over both batch elements,
and a partial output projection (its 128 rows of wo). Host sums the 8
partials and adds bo.

Device layout (per core):
  xT      (1024, 4096)  x transposed, tok = b*2048 + s
  QT/KT   (128, 4096)   q^T/k^T, partitions = [h0 dk(64) | h1 dk(64)]
  V       (128, 32, 130) per 128-tok chunk: [V_h0(64) | 1 | V_h1(64) | 1]
  scores  computed transposed: S^T (k on partitions, q on free) so the
          softmax denominator falls out of the PV matmul via the ones
          column, and O^T is produced in exactly the layout the output
          projection needs as its stationary operand.
  bias    B_h (2048, 2048) host-precomputed (symmetric), DMA-streamed.
"""

import ml_dtypes
import numpy as np
from contextlib import ExitStack

import concourse.bass as bass
import concourse.bacc as bacc
import concourse.mybir as mybir
import concourse.tile as tile
from concourse.bass_utils import run_bass_kernel_spmd
from concourse.masks import make_identity

F32 = mybir.dt.float32
F32R = mybir.dt.float32r
BF16 = mybir.dt.bfloat16
AF = mybir.ActivationFunctionType

# float32r: single-pass fp32 matmul mode (1 cycle/row vs 4 for fp32),
# ~1.2e-4 relative rounding on HW. Toggle for a full-precision fallback.
USE_F32R = True
MMDT = F32R if USE_F32R else F32

N_CORES = 8
B, S, D = 2, 2048, 1024
H, DK = 16, 64
TOK = B * S              # 4096
NQ = 512                 # q-chunk (matmul free dim)
NQC = S // NQ            # 4 q-chunks per batch
NJ = S // 128            # 16 k-chunks per batch
DCH = D // 128           # 8 contraction chunks

# Set by test.py for profiling; harness leaves these untouched.
TRACE = False
TRACE_DIR = None

_CACHED_NC = None


def _body(ctx: ExitStack, tc, aps):
    nc = tc.nc
    xT, wq, wk, wv, bq, bk, bv, wo, B0, B1, out = (
        aps["xT"], aps["wq"], aps["wk"], aps["wv"], aps["bq"], aps["bk"],
        aps["bv"], aps["wo"], aps["B0"], aps["B1"], aps["out"])
    Bh = [B0, B1]

    const = ctx.enter_context(tc.tile_pool(name="const", bufs=1))
    persist = ctx.enter_context(tc.tile_pool(name="persist", bufs=1))
    spool = ctx.enter_context(tc.tile_pool(name="spsum", bufs=2, space="PSUM"))
    opool = ctx.enter_context(tc.tile_pool(name="opsum", bufs=4, space="PSUM"))

    # ---- constants ----
    ident = const.tile([128, 128], F32, tag="id")
    make_identity(nc, ident[:])
    w_sb = {}
    for name, ap in (("q", wq), ("k", wk), ("v", wv)):
        t = const.tile([128, DCH, 128], MMDT, tag=f"w{name}")
        nc.sync.dma_start(t[:], ap.rearrange("(c p) m -> p c m", p=128))
        w_sb[name] = t
    b_sb = {}
    for name, ap in (("q", bq), ("k", bk), ("v", bv)):
        t = const.tile([128, 1], F32, tag=f"b{name}")
        nc.sync.dma_start(t[:], ap[:])
        b_sb[name] = t
    # wo split into the two 64-row halves so both out-proj matmuls run at
    # partition base 0.
    wo0 = const.tile([64, D], MMDT, tag="wo0")
    wo1 = const.tile([64, D], MMDT, tag="wo1")
    nc.sync.dma_start(wo0[:], wo[0:64, :])
    nc.sync.dma_start(wo1[:], wo[64:128, :])
    # ones row at partition 64 for broadcasting the softmax reciprocal
    # (gpsimd partition_broadcast is broken on HW via this exec path)
    ones64 = const.tile([65, 64], F32, tag="ones64")
    nc.vector.memset(ones64[:], 1.0)
    # fp32 ones staged for the f32r ones-columns of Vb (memset can't
    # write f32r; ACT copy is a rounding producer)
    ones_f32 = const.tile([128, 64], F32, tag="ones_f32")
    nc.vector.memset(ones_f32[:], 1.0)

    # ---- persistent activations ----
    QT = persist.tile([128, TOK], MMDT, tag="QT")
    KT = persist.tile([128, TOK], MMDT, tag="KT")
    Vb = persist.tile([128, TOK // 128, 130], MMDT, tag="Vb")
    OT = [persist.tile([65, TOK], MMDT, tag=f"OT{h}", name=f"OT{h}")
          for h in range(2)]

    # ones columns of V_ext (positions 64 and 129 of each 130-stripe)
    nc.scalar.copy(
        Vb[:].rearrange("p t (g x) -> p t g x", g=2)[:, :, :, 64:65],
        ones_f32[:].rearrange("p (t g x) -> p t g x", t=TOK // 128, g=2))

    # ---- phases 1-2: QKV projections + V transpose (scoped pools) ----
    with tc.tile_pool(name="xin", bufs=2) as xpool, \
         tc.tile_pool(name="vtp", bufs=1) as vtpool:
        VT = vtpool.tile([128, TOK], F32, tag="VT")
        xT_r = xT.rearrange("(c p) n -> p c n", p=128)
        for t in range(TOK // NQ):
            xt = xpool.tile([128, DCH, NQ], MMDT, tag="x")
            nc.sync.dma_start(xt[:], xT_r[:, :, t * NQ:(t + 1) * NQ])
            for name, dst in (("q", QT), ("k", KT), ("v", VT)):
                ps = opool.tile([128, NQ], F32, tag="o", name="ps_proj")
                for d in range(DCH):
                    nc.tensor.matmul(ps[:], w_sb[name][:, d, :], xt[:, d, :],
                                     start=(d == 0), stop=(d == DCH - 1))
                nc.vector.tensor_scalar_add(
                    dst[:, t * NQ:(t + 1) * NQ], ps[:], b_sb[name][:])

        # V transpose into (tok, hd) chunks
        for u in range(TOK // 128):
            pt = opool.tile([128, 128], F32, tag="o", name="pt_tr")
            nc.tensor.transpose(pt[:], VT[:, u * 128:(u + 1) * 128], ident[:])
            nc.scalar.copy(
                Vb[:, u, :].rearrange("p (g x) -> p g x", g=2)[:, :, 0:64],
                pt[:].rearrange("p (g x) -> p g x", g=2))

    # ---- phase 3: attention ----
    bpool = ctx.enter_context(tc.tile_pool(name="bin", bufs=3))
    ppool = ctx.enter_context(tc.tile_pool(name="pexp", bufs=6))
    small = ctx.enter_context(tc.tile_pool(name="small", bufs=2))
    rec = [small.tile([65, TOK], F32, tag="rec", name=f"rec{h}")
           for h in range(2)]
    B_r = [Bh[h].rearrange("(j p) q -> p j q", p=128) for h in range(2)]
    for qc in range(NQC):
        O_ps = [[opool.tile([65, NQ], F32, tag="o", name=f"O_ps{hh}{bb}")
                 for bb in range(2)] for hh in range(2)]
        for jq in range(4):  # quarter-stripes of 4 k-chunks
            # both heads interleaved per k-chunk -> one wide DVE add later
            bstr = bpool.tile([128, 4, 2, NQ], BF16, tag="b")
            for hh in range(2):
                nc.sync.dma_start(
                    bstr[:, :, hh, :],
                    B_r[hh][:, jq * 4:(jq + 1) * 4, qc * NQ:(qc + 1) * NQ])
            for b in range(2):
                q0 = b * S + qc * NQ
                for ji in range(4):
                    j = jq * 4 + ji
                    k0 = b * S + j * 128
                    # head-packed scores: h0 in PE rows 0-63, h1 in rows
                    # 64-127, issued adjacently
                    sps = spool.tile([128, 2 * NQ], F32, tag="s")
                    for hh in range(2):
                        nc.tensor.matmul(
                            sps[:, hh * NQ:(hh + 1) * NQ],
                            KT[64 * hh:64 * hh + 64, k0:k0 + 128],
                            QT[64 * hh:64 * hh + 64, q0:q0 + NQ],
                            start=True, stop=True)
                    # lag bias on DVE (PSUM in-place, both heads at once)
                    nc.vector.tensor_add(
                        sps[:], sps[:],
                        bstr[:, ji, :, :].rearrange("p g q -> p (g q)"))
                    pe = ppool.tile([128, 2 * NQ], MMDT, tag="p")
                    nc.scalar.activation(pe[:], sps[:], AF.Exp)
                    for hh in range(2):
                        nc.tensor.matmul(
                            O_ps[hh][b][:],
                            Vb[:, b * NJ + j, 65 * hh:65 * hh + 65],
                            pe[:, hh * NQ:(hh + 1) * NQ],
                            start=(j == 0), stop=(j == NJ - 1))
        for hh in range(2):
            for b in range(2):
                q0 = b * S + qc * NQ
                # stash unnormalized O^T + denominator row; normalize later
                nc.scalar.copy(OT[hh][:, q0:q0 + NQ], O_ps[hh][b][:])
                nc.vector.reciprocal(rec[hh][64:65, q0:q0 + NQ],
                                     OT[hh][64:65, q0:q0 + NQ].bitcast(F32))

    # ---- phase 3b/4: normalize + output projection, pipelined ----
    for c in range(TOK // NQ):
        sl = slice(c * NQ, (c + 1) * NQ)
        for h in range(2):
            R_ps = opool.tile([64, NQ], F32, tag="o", name="R_ps")
            nc.tensor.matmul(R_ps[:], ones64[64:65, :], rec[h][64:65, sl],
                             start=True, stop=True)
            nc.vector.tensor_mul(OT[h][0:64, sl], OT[h][0:64, sl], R_ps[:])
        for u in range(4 * c, 4 * c + 4):
            ps = spool.tile([128, 2 * NQ], F32, tag="s")
            for half in range(2):
                osl = slice(half * NQ, (half + 1) * NQ)
                nc.tensor.matmul(ps[:, osl],
                                 OT[0][0:64, u * 128:(u + 1) * 128],
                                 wo0[:, osl], start=True, stop=False)
                nc.tensor.matmul(ps[:, osl],
                                 OT[1][0:64, u * 128:(u + 1) * 128],
                                 wo1[:, osl], start=False, stop=True)
            osb = ppool.tile([128, 2 * NQ], F32, tag="osb")
            nc.scalar.copy(osb[:], ps[:])
            nc.sync.dma_start(out[u * 128:(u + 1) * 128, :], osb[:])


def build_program():
    nc = bacc.Bacc("TRN2", target_bir_lowering=False, debug=False,
                   enable_asserts=False, num_devices=N_CORES)
    aps = {}
    specs = [
        ("xT", (D, TOK), MMDT), ("wq", (D, 128), MMDT), ("wk", (D, 128), MMDT),
        ("wv", (D, 128), MMDT), ("bq", (128, 1), F32), ("bk", (128, 1), F32),
        ("bv", (128, 1), F32), ("wo", (128, D), MMDT), ("B0", (S, S), BF16),
        ("B1", (S, S), BF16),
    ]
    for name, shape, dt in specs:
        aps[name] = nc.dram_tensor(name, shape, dt, kind="ExternalInput").ap()
    aps["out"] = nc.dram_tensor("out", (TOK, D), F32,
                                kind="ExternalOutput").ap()
    with tile.TileContext(nc) as tc:
        with ExitStack() as ctx:
            _body(ctx, tc, aps)
    nc.compile()
    return nc


def _get_nc():
    global _CACHED_NC
    if _CACHED_NC is None:
        _CACHED_NC = build_program()
    return _CACHED_NC


def _host_prep(x, lag, wq, bq, wk, bk, wv, bv, wo, bo, lag_bias):
    x = np.ascontiguousarray(np.asarray(x, dtype=np.float32))
    lag = np.asarray(lag).astype(np.int64)
    xT = np.ascontiguousarray(x.reshape(TOK, D).T)
    ld = np.abs(lag[:, None] - lag[None, :]).astype(np.int64)
    lag_bias = np.asarray(lag_bias, dtype=np.float32)
    scale = np.float32(1.0 / np.sqrt(DK))
    wq = np.asarray(wq, dtype=np.float32) * scale
    bq = np.asarray(bq, dtype=np.float32) * scale
    in_maps = []
    for c in range(N_CORES):
        sl = slice(c * 128, (c + 1) * 128)
        in_maps.append({
            "xT": xT,
            "wq": np.ascontiguousarray(wq[:, sl]),
            "wk": np.ascontiguousarray(np.asarray(wk, np.float32)[:, sl]),
            "wv": np.ascontiguousarray(np.asarray(wv, np.float32)[:, sl]),
            "bq": np.ascontiguousarray(bq[sl].reshape(128, 1)),
            "bk": np.ascontiguousarray(np.asarray(bk, np.float32)[sl].reshape(128, 1)),
            "bv": np.ascontiguousarray(np.asarray(bv, np.float32)[sl].reshape(128, 1)),
            "wo": np.ascontiguousarray(np.asarray(wo, np.float32)[sl, :]),
            "B0": np.ascontiguousarray(
                lag_bias[2 * c][ld].astype(ml_dtypes.bfloat16)),
            "B1": np.ascontiguousarray(
                lag_bias[2 * c + 1][ld].astype(ml_dtypes.bfloat16)),
        })
    return in_maps


def kernel(x, lag, wq, bq, wk, bk, wv, bv, wo, bo, lag_bias):
    nc = _get_nc()
    in_maps = _host_prep(x, lag, wq, bq, wk, bk, wv, bv, wo, bo, lag_bias)
    kwargs = {}
    if TRACE:
        kwargs = dict(trace=True, tmpdir=TRACE_DIR)
    res = run_bass_kernel_spmd(nc, in_maps, core_ids=list(range(N_CORES)),
                               **kwargs)
    if TRACE:
        print(f"HW exec time: {res.exec_time_ns} ns")
    total = res.results[0]["out"].astype(np.float32)
    for c in range(1, N_CORES):
        total += res.results[c]["out"]
    total += np.asarray(bo, dtype=np.float32)[None, :]
    return total.reshape(B, S, D)


# revision 29
# speedup vs baseline: 1.0674x; 1.0160x over previous
"""MultiHeadSelfAttentionWithLagBias on 8 TRN2 NeuronCores.

Sharding: tensor-parallel over heads — 16 heads / 8 cores = 2 heads per
core. Each core computes QKV projections for its head slice (full x),
attention with the lag bias for its 2 heads over both batch elements,
and a partial output projection (its 128 rows of wo). Host sums the 8
partials and adds bo.

Device layout (per core):
  xT      (1024, 4096)  x transposed, tok = b*2048 + s
  QT/KT   (128, 4096)   q^T/k^T, partitions = [h0 dk(64) | h1 dk(64)]
  V       (128, 32, 130) per 128-tok chunk: [V_h0(64) | 1 | V_h1(64) | 1]
  scores  computed transposed: S^T (k on partitions, q on free) so the
          softmax denominator falls out of the PV matmul via the ones
          column, and O^T is produced in exactly the layout the output
          projection needs as its stationary operand.
  bias    B_h (2048, 2048) host-precomputed (symmetric), DMA-streamed.
"""

import ml_dtypes
import numpy as np
from contextlib import ExitStack

import concourse.bass as bass
import concourse.bacc as bacc
import concourse.mybir as mybir
import concourse.tile as tile
from concourse.bass_utils import run_bass_kernel_spmd
from concourse.masks import make_identity

F32 = mybir.dt.float32
F32R = mybir.dt.float32r
BF16 = mybir.dt.bfloat16
AF = mybir.ActivationFunctionType

# float32r: single-pass fp32 matmul mode (1 cycle/row vs 4 for fp32),
# ~1.2e-4 relative rounding on HW. Toggle for a full-precision fallback.
USE_F32R = True
MMDT = F32R if USE_F32R else F32

N_CORES = 8
B, S, D = 2, 2048, 1024
H, DK = 16, 64
TOK = B * S              # 4096
NQ = 512                 # q-chunk (matmul free dim)
NQC = S // NQ            # 4 q-chunks per batch
NJ = S // 128            # 16 k-chunks per batch
DCH = D // 128           # 8 contraction chunks

# Set by test.py for profiling; harness leaves these untouched.
TRACE = False
TRACE_DIR = None

_CACHED_NC = None


def _body(ctx: ExitStack, tc, aps):
    nc = tc.nc
    xT, wq, wk, wv, bq, bk, bv, wo, B0, B1, out = (
        aps["xT"], aps["wq"], aps["wk"], aps["wv"], aps["bq"], aps["bk"],
        aps["bv"], aps["wo"], aps["B0"], aps["B1"], aps["out"])
    Bh = [B0, B1]

    const = ctx.enter_context(tc.tile_pool(name="const", bufs=1))
    persist = ctx.enter_context(tc.tile_pool(name="persist", bufs=1))
    spool = ctx.enter_context(tc.tile_pool(name="spsum", bufs=2, space="PSUM"))
    opool = ctx.enter_context(tc.tile_pool(name="opsum", bufs=4, space="PSUM"))

    # ---- constants ----
    ident = const.tile([128, 128], F32, tag="id")
    make_identity(nc, ident[:])
    w_sb = {}
    for name, ap in (("q", wq), ("k", wk), ("v", wv)):
        t = const.tile([128, DCH, 128], MMDT, tag=f"w{name}")
        nc.sync.dma_start(t[:], ap.rearrange("(c p) m -> p c m", p=128))
        w_sb[name] = t
    b_sb = {}
    for name, ap in (("q", bq), ("k", bk), ("v", bv)):
        t = const.tile([128, 1], F32, tag=f"b{name}")
        nc.sync.dma_start(t[:], ap[:])
        b_sb[name] = t
    # wo split into the two 64-row halves so both out-proj matmuls run at
    # partition base 0.
    wo0 = const.tile([64, D], MMDT, tag="wo0")
    wo1 = const.tile([64, D], MMDT, tag="wo1")
    nc.sync.dma_start(wo0[:], wo[0:64, :])
    nc.sync.dma_start(wo1[:], wo[64:128, :])
    # ones row at partition 64 for broadcasting the softmax reciprocal
    # (gpsimd partition_broadcast is broken on HW via this exec path)
    ones64 = const.tile([65, 64], F32, tag="ones64")
    nc.vector.memset(ones64[:], 1.0)
    # fp32 ones staged for the f32r ones-columns of Vb (memset can't
    # write f32r; ACT copy is a rounding producer)
    ones_f32 = const.tile([128, 64], F32, tag="ones_f32")
    nc.vector.memset(ones_f32[:], 1.0)

    # ---- persistent activations ----
    QT = persist.tile([128, TOK], MMDT, tag="QT")
    KT = persist.tile([128, TOK], MMDT, tag="KT")
    Vb = persist.tile([128, TOK // 128, 130], MMDT, tag="Vb")
    OT = [persist.tile([65, TOK], MMDT, tag=f"OT{h}", name=f"OT{h}")
          for h in range(2)]

    # ones columns of V_ext (positions 64 and 129 of each 130-stripe)
    nc.scalar.copy(
        Vb[:].rearrange("p t (g x) -> p t g x", g=2)[:, :, :, 64:65],
        ones_f32[:].rearrange("p (t g x) -> p t g x", t=TOK // 128, g=2))

    # ---- phases 1-2: QKV projections + V transpose (scoped pools) ----
    with tc.tile_pool(name="xin", bufs=3) as xpool, \
         tc.tile_pool(name="vtp", bufs=1) as vtpool:
        VT = vtpool.tile([128, TOK], F32, tag="VT")
        xT_r = xT.rearrange("(c p) n -> p c n", p=128)
        for t in range(TOK // NQ):
            xt = xpool.tile([128, DCH, NQ], MMDT, tag="x")
            nc.sync.dma_start(xt[:], xT_r[:, :, t * NQ:(t + 1) * NQ])
            for name, dst in (("q", QT), ("k", KT), ("v", VT)):
                ps = opool.tile([128, NQ], F32, tag="o", name="ps_proj")
                for d in range(DCH):
                    nc.tensor.matmul(ps[:], w_sb[name][:, d, :], xt[:, d, :],
                                     start=(d == 0), stop=(d == DCH - 1))
                nc.vector.tensor_scalar_add(
                    dst[:, t * NQ:(t + 1) * NQ], ps[:], b_sb[name][:])

        # V transpose into (tok, hd) chunks
        for u in range(TOK // 128):
            pt = opool.tile([128, 128], F32, tag="o", name="pt_tr")
            nc.tensor.transpose(pt[:], VT[:, u * 128:(u + 1) * 128], ident[:])
            nc.scalar.copy(
                Vb[:, u, :].rearrange("p (g x) -> p g x", g=2)[:, :, 0:64],
                pt[:].rearrange("p (g x) -> p g x", g=2))

    # ---- phase 3: attention ----
    bpool = ctx.enter_context(tc.tile_pool(name="bin", bufs=3))
    ppool = ctx.enter_context(tc.tile_pool(name="pexp", bufs=6))
    small = ctx.enter_context(tc.tile_pool(name="small", bufs=2))
    rec = [small.tile([65, TOK], F32, tag="rec", name=f"rec{h}")
           for h in range(2)]
    B_r = [Bh[h].rearrange("(j p) q -> p j q", p=128) for h in range(2)]
    for qc in range(NQC):
        O_ps = [[opool.tile([65, NQ], F32, tag="o", name=f"O_ps{hh}{bb}")
                 for bb in range(2)] for hh in range(2)]
        for jq in range(4):  # quarter-stripes of 4 k-chunks
            # both heads interleaved per k-chunk -> one wide DVE add later
            bstr = bpool.tile([128, 4, 2, NQ], BF16, tag="b")
            for hh in range(2):
                nc.sync.dma_start(
                    bstr[:, :, hh, :],
                    B_r[hh][:, jq * 4:(jq + 1) * 4, qc * NQ:(qc + 1) * NQ])
            for b in range(2):
                q0 = b * S + qc * NQ
                for ji in range(4):
                    j = jq * 4 + ji
                    k0 = b * S + j * 128
                    # head-packed scores: h0 in PE rows 0-63, h1 in rows
                    # 64-127, issued adjacently
                    sps = spool.tile([128, 2 * NQ], F32, tag="s")
                    for hh in range(2):
                        nc.tensor.matmul(
                            sps[:, hh * NQ:(hh + 1) * NQ],
                            KT[64 * hh:64 * hh + 64, k0:k0 + 128],
                            QT[64 * hh:64 * hh + 64, q0:q0 + NQ],
                            start=True, stop=True)
                    # lag bias on DVE (PSUM in-place, both heads at once)
                    nc.vector.tensor_add(
                        sps[:], sps[:],
                        bstr[:, ji, :, :].rearrange("p g q -> p (g q)"))
                    pe = ppool.tile([128, 2 * NQ], MMDT, tag="p")
                    nc.scalar.activation(pe[:], sps[:], AF.Exp)
                    for hh in range(2):
                        nc.tensor.matmul(
                            O_ps[hh][b][:],
                            Vb[:, b * NJ + j, 65 * hh:65 * hh + 65],
                            pe[:, hh * NQ:(hh + 1) * NQ],
                            start=(j == 0), stop=(j == NJ - 1))
        for hh in range(2):
            for b in range(2):
                q0 = b * S + qc * NQ
                # stash unnormalized O^T + denominator row; normalize later
                nc.scalar.copy(OT[hh][:, q0:q0 + NQ], O_ps[hh][b][:])
                nc.vector.reciprocal(rec[hh][64:65, q0:q0 + NQ],
                                     OT[hh][64:65, q0:q0 + NQ].bitcast(F32))

    # ---- phase 3b/4: normalize + output projection, pipelined ----
    for c in range(TOK // NQ):
        sl = slice(c * NQ, (c + 1) * NQ)
        for h in range(2):
            R_ps = opool.tile([64, NQ], F32, tag="o", name="R_ps")
            nc.tensor.matmul(R_ps[:], ones64[64:65, :], rec[h][64:65, sl],
                             start=True, stop=True)
            nc.vector.tensor_mul(OT[h][0:64, sl], OT[h][0:64, sl], R_ps[:])
        for u in range(4 * c, 4 * c + 4):
            ps = spool.tile([128, 2 * NQ], F32, tag="s")
            for half in range(2):
                osl = slice(half * NQ, (half + 1) * NQ)
                nc.tensor.matmul(ps[:, osl],
                                 OT[0][0:64, u * 128:(u + 1) * 128],
                                 wo0[:, osl], start=True, stop=False)
                nc.tensor.matmul(ps[:, osl],
                                 OT[1][0:64, u * 128:(u + 1) * 128],
                                 wo1[:, osl], start=False, stop=True)
            osb = ppool.tile([128, 2 * NQ], F32, tag="osb")
            # alternate engines so the drain isn't serialized on ScalarE
            if u % 2 == 0:
                nc.scalar.copy(osb[:], ps[:])
            else:
                nc.vector.tensor_copy(osb[:], ps[:])
            nc.sync.dma_start(out[u * 128:(u + 1) * 128, :], osb[:])


def build_program():
    nc = bacc.Bacc("TRN2", target_bir_lowering=False, debug=False,
                   enable_asserts=False, num_devices=N_CORES)
    aps = {}
    specs = [
        ("xT", (D, TOK), MMDT), ("wq", (D, 128), MMDT), ("wk", (D, 128), MMDT),
        ("wv", (D, 128), MMDT), ("bq", (128, 1), F32), ("bk", (128, 1), F32),
        ("bv", (128, 1), F32), ("wo", (128, D), MMDT), ("B0", (S, S), BF16),
        ("B1", (S, S), BF16),
    ]
    for name, shape, dt in specs:
        aps[name] = nc.dram_tensor(name, shape, dt, kind="ExternalInput").ap()
    aps["out"] = nc.dram_tensor("out", (TOK, D), F32,
                                kind="ExternalOutput").ap()
    with tile.TileContext(nc) as tc:
        with ExitStack() as ctx:
            _body(ctx, tc, aps)
    nc.compile()
    return nc


def _get_nc():
    global _CACHED_NC
    if _CACHED_NC is None:
        _CACHED_NC = build_program()
    return _CACHED_NC


def _host_prep(x, lag, wq, bq, wk, bk, wv, bv, wo, bo, lag_bias):
    x = np.ascontiguousarray(np.asarray(x, dtype=np.float32))
    lag = np.asarray(lag).astype(np.int64)
    xT = np.ascontiguousarray(x.reshape(TOK, D).T)
    ld = np.abs(lag[:, None] - lag[None, :]).astype(np.int64)
    lag_bias = np.asarray(lag_bias, dtype=np.float32)
    scale = np.float32(1.0 / np.sqrt(DK))
    wq = np.asarray(wq, dtype=np.float32) * scale
    bq = np.asarray(bq, dtype=np.float32) * scale
    in_maps = []
    for c in range(N_CORES):
        sl = slice(c * 128, (c + 1) * 128)
        in_maps.append({
            "xT": xT,
            "wq": np.ascontiguousarray(wq[:, sl]),
            "wk": np.ascontiguousarray(np.asarray(wk, np.float32)[:, sl]),
            "wv": np.ascontiguousarray(np.asarray(wv, np.float32)[:, sl]),
            "bq": np.ascontiguousarray(bq[sl].reshape(128, 1)),
            "bk": np.ascontiguousarray(np.asarray(bk, np.float32)[sl].reshape(128, 1)),
            "bv": np.ascontiguousarray(np.asarray(bv, np.float32)[sl].reshape(128, 1)),
            "wo": np.ascontiguousarray(np.asarray(wo, np.float32)[sl, :]),
            "B0": np.ascontiguousarray(
                lag_bias[2 * c][ld].astype(ml_dtypes.bfloat16)),
            "B1": np.ascontiguousarray(
                lag_bias[2 * c + 1][ld].astype(ml_dtypes.bfloat16)),
        })
    return in_maps


def kernel(x, lag, wq, bq, wk, bk, wv, bv, wo, bo, lag_bias):
    nc = _get_nc()
    in_maps = _host_prep(x, lag, wq, bq, wk, bk, wv, bv, wo, bo, lag_bias)
    kwargs = {}
    if TRACE:
        kwargs = dict(trace=True, tmpdir=TRACE_DIR)
    res = run_bass_kernel_spmd(nc, in_maps, core_ids=list(range(N_CORES)),
                               **kwargs)
    if TRACE:
        print(f"HW exec time: {res.exec_time_ns} ns")
    total = res.results[0]["out"].astype(np.float32)
    for c in range(1, N_CORES):
        total += res.results[c]["out"]
    total += np.asarray(bo, dtype=np.float32)[None, :]
    return total.reshape(B, S, D)
